# revision 2
# baseline (speedup 1.0000x reference)
"""Trainium2 Bass kernel for the MFCA channel-attention module.

  q = x_RGB.reshape(B, C, N); k = v = x.reshape(B, C, N)
  energy    = q @ k^T                          (B, C, C)
  attention = softmax(max(energy, -1) - energy)   over last axis
  out       = delta * (attention @ v) + x

Numerically, softmax(max - energy) == softmax(-energy); the stable form is
p = exp(min_row(energy) - energy), attention = p / rowsum(p).

Sharding: data-parallel over batch B=16 across 8 NeuronCores (2 per core).

v6 (arrival-ordered PE stream, fine-grained loads, streamed stores):
  - loads split into [128, 512] pieces (piece-set cadence ~5.5us) so the
    transpose/MM1 stream tracks DMA arrival instead of waiting on whole
    1024-column quarters; x8 fp8 cast copies alternate ACT/DVE per piece.
  - PE emission is strictly arrival-ordered: T(pair p) never precedes
    ready MM1(p-1) work at a piece boundary, so the in-order PE queue
    cannot head-of-line-block on a load.  Within a piece, MM1(p-1) is
    interleaved into T(p) to hide transpose-stage drain latency.
  - the previous batch's MM2 blocks are the fillers for load-stall gaps
    (2 per pair iteration), emitted *before* T(p) so they run while T(p)
    waits on data.
  - MM2 stores stream per (i, 2-block) [128, 1024] chunk instead of per
    n-half, killing the store-only tail.
"""

from contextlib import ExitStack

import numpy as np

import concourse.bass as bass
import concourse.tile as tile
from concourse import bacc, mybir
from concourse.bass_utils import run_bass_kernel_spmd
from concourse.masks import make_identity

N_CORES = 8
B, C, H, W = 16, 512, 64, 64
N = H * W  # 4096
BS = B // N_CORES  # batches per core

F32 = mybir.dt.float32
BF16 = mybir.dt.bfloat16
FP8 = mybir.dt.float8e4

DR = mybir.MatmulPerfMode.DoubleRow


def build_nc(bs=BS, c=C, n=N):
    """Build the single-core Bass program (SPMD across all cores)."""
    nc = bacc.Bacc(None, target_bir_lowering=False, debug=False)

    x_d = nc.dram_tensor("x", [bs, c, n], F32, kind="ExternalInput")
    q_d = nc.dram_tensor("x_RGB", [bs, c, n], F32, kind="ExternalInput")
    d_d = nc.dram_tensor("delta", [128, 1], F32, kind="ExternalInput")
    o_d = nc.dram_tensor("out", [bs, c, n], BF16, kind="ExternalOutput")

    nct = c // 128  # channel chunks (i-tiles / j-tiles)
    npr = n // 256  # DoubleRow n-pairs (16)
    nnb = n // 512  # n-blocks in the output matmul (8)
    PC = 512  # load piece columns
    npc = n // PC  # pieces per chunk row (8)

    with tile.TileContext(nc) as tc, ExitStack() as ctx:
        pxb = ctx.enter_context(tc.tile_pool(name="pxb", bufs=68))
        px8 = ctx.enter_context(tc.tile_pool(name="px8", bufs=2))
        pq8 = ctx.enter_context(tc.tile_pool(name="pq8", bufs=2))
        pqt = ctx.enter_context(tc.tile_pool(name="pqt", bufs=4))
        pp = ctx.enter_context(tc.tile_pool(name="pp", bufs=6))
        ppt = ctx.enter_context(tc.tile_pool(name="ppt", bufs=5))
        pout = ctx.enter_context(tc.tile_pool(name="pout", bufs=8))
        psml = ctx.enter_context(tc.tile_pool(name="psml", bufs=8))
        pone = ctx.enter_context(tc.tile_pool(name="pone", bufs=1))
        pe_pool = ctx.enter_context(tc.tile_pool(name="pe", bufs=4, space="PSUM"))
        ptr_pool = ctx.enter_context(tc.tile_pool(name="ptr", bufs=2, space="PSUM"))
        pu_pool = ctx.enter_context(tc.tile_pool(name="pu", bufs=2, space="PSUM"))

        ident8 = pone.tile([128, 128], FP8)
        make_identity(nc, ident8[:])
        delta_sb = pone.tile([128, 1], F32)
        nc.sync.dma_start(out=delta_sb[:], in_=d_d[:])

        def emit_loads(b):
            """Load one batch in [128, PC] pieces, piece-set major, so data
            arrives in exactly the order the transpose stream consumes it.
            x lands as bf16 (residual + fp8 cast source), x_RGB as fp8."""
            xbs = [[None] * npc for _ in range(nct)]
            x8 = px8.tile([128, nct, n], FP8, name="x8", tag="x8")
            q8 = pq8.tile([128, nct, n], FP8, name="q8", tag="q8")

            for pi in range(npc):
                cs = slice(pi * PC, (pi + 1) * PC)
                for k in range(nct):
                    xb = pxb.tile([128, PC], BF16)
                    nc.gpsimd.dma_start(
                        out=xb[:], in_=x_d[b, 128 * k : 128 * (k + 1), cs]
                    )
                    nc.gpsimd.dma_start(
                        out=q8[:, k, cs], in_=q_d[b, 128 * k : 128 * (k + 1), cs]
                    )
                    if (pi * nct + k) % 2 == 0:
                        nc.scalar.copy(out=x8[:, k, cs], in_=xb[:])
                    else:
                        nc.vector.tensor_copy(out=x8[:, k, cs], in_=xb[:])
                    xbs[k][pi] = xb
            return xbs, x8, q8

        def drain_eng(l):
            return "v" if l % 8 < 3 else "s"  # 12 DVE / 20 ACT per batch

        def t_stream(p, nxt, q8, x8):
            """Generator yielding after each fp8 PE transpose of pair p, so
            the caller can interleave MM1 matmuls of the previous pair."""
            for li in range(2):
                nt = 2 * p + li
                ns = slice(128 * nt, 128 * (nt + 1))
                stage = ptr_pool.tile(
                    [128, 2, c, 2], FP8, name="tstage", tag="stage"
                )
                for cc in range(nct):
                    nc.tensor.transpose(
                        stage[:, 0, 128 * cc : 128 * (cc + 1), 0],
                        q8[:, cc, ns],
                        ident8[:],
                    )
                    yield
                    nc.tensor.transpose(
                        stage[:, 1, 128 * cc : 128 * (cc + 1), 0],
                        x8[:, cc, ns],
                        ident8[:],
                    )
                    yield
                if drain_eng(nt) == "v":
                    nc.vector.tensor_copy(out=nxt[:, li, :, :], in_=stage[:, :, :, 0])
                else:
                    nc.scalar.copy(out=nxt[:, li, :, :], in_=stage[:, :, :, 0])

        def emit_mm1(p, es, qxt, ts):
            """4 DoubleRow matmuls (one per i-tile) for n-pair p; if ts is
            given, interleave the next pair's transposes 4-per-matmul."""
            for i in range(nct):
                nc.tensor.matmul(
                    es[i][:],
                    qxt[:, :, 0, 128 * i : 128 * (i + 1)],
                    qxt[:, :, 1, :],
                    start=(p == 0),
                    stop=(p == npr - 1),
                    perf_mode=DR,
                )
                if ts is not None:
                    for _ in range(4):
                        next(ts, None)
            if ts is not None:
                for _ in ts:
                    pass

        def emit_softmax(i, es):
            e = es[i]
            m = psml.tile([128, 1], F32)
            nc.vector.tensor_reduce(
                m[:], e[:], axis=mybir.AxisListType.X, op=mybir.AluOpType.min
            )
            p_t = pp.tile([128, c], BF16)
            z = psml.tile([128, 1], F32)
            nc.scalar.activation(
                out=p_t[:],
                in_=e[:],
                func=mybir.ActivationFunctionType.Exp,
                bias=m[:],
                scale=-1.0,
                accum_out=z[:],
            )
            zi = psml.tile([128, 1], F32)
            nc.vector.reciprocal(zi[:], z[:])
            s = psml.tile([128, 1], F32)
            nc.vector.tensor_scalar_mul(s[:], zi[:], delta_sb[:])  # delta / Z
            # Fold delta/Z into P here so MM2 needs no per-block scaling.
            ps = pp.tile([128, c], FP8)
            nc.vector.tensor_scalar_mul(ps[:], p_t[:], s[:])
            # P'^T via fp8 PE transposes (step-2 stage), drained to
            # [128, jt, 128] so the MM2 DoubleRow stationary is a jt-pair
            # slice.
            pstage = ptr_pool.tile(
                [128, nct, 128, 2], FP8, name="pstage", tag="stage"
            )
            for jt in range(nct):
                nc.tensor.transpose(
                    pstage[:, jt, :, 0],
                    ps[:, 128 * jt : 128 * (jt + 1)],
                    ident8[:],
                )
            pt = ppt.tile([128, nct, 128], FP8)
            nc.scalar.copy(out=pt[:], in_=pstage[:, :, :, 0])
            return pt

        def mm2_stream(b, sm, x8, xbs):
            """Generator of MM2 blocks: one (i, nb) output block per step;
            epilogue adds bf16 x; stores stream per (i, 2-block) chunk."""
            for nbp in range(nnb // 2):
                for i in range(nct):
                    pt = sm[i]
                    ob = pout.tile([128, 1024], BF16, name=f"ob{nbp}_{i}", tag="ob")
                    for s in range(2):
                        gnb = 2 * nbp + s
                        ns = slice(512 * gnb, 512 * (gnb + 1))
                        u = pu_pool.tile([128, 512], F32, name="u", tag="u")
                        for jp in range(2):
                            nc.tensor.matmul(
                                u[:],
                                pt[:, 2 * jp : 2 * jp + 2, :],
                                x8[:, 2 * jp : 2 * jp + 2, ns],
                                start=(jp == 0),
                                stop=(jp == 1),
                                perf_mode=DR,
                            )
                        nc.vector.tensor_add(
                            ob[:, 512 * s : 512 * (s + 1)],
                            u[:],
                            xbs[i][gnb][:],
                        )
                        yield
                    nc.sync.dma_start(
                        out=o_d[b, 128 * i : 128 * (i + 1), 1024 * nbp : 1024 * (nbp + 1)],
                        in_=ob[:],
                    )

        def emit_batch_front(b, mm2):
            """Loads, transposes, energy matmuls, and softmax for one batch;
            the previous batch's MM2 blocks fill load-stall gaps.  Emission
            is strictly arrival-ordered: at a piece boundary (pair p needs a
            new piece-set) all ready work (MM1(p-1), fillers) is emitted
            BEFORE T(p) so the in-order PE queue never blocks on a load."""
            xbs, x8, q8 = emit_loads(b)
            es = [
                pe_pool.tile([128, c], F32, name=f"e{i}", tag="e") for i in range(nct)
            ]
            prev = None
            for p in range(npr):
                qxt = pqt.tile([128, 2, 2, c], FP8, name="qxt", tag="qxt")
                ts = t_stream(p, qxt, q8, x8)
                boundary = (p % 2 == 0)  # pair p starts a new piece-set
                if p > 0 and not boundary:
                    emit_mm1(p - 1, es, prev, ts)  # interleave into T(p)
                else:
                    if p > 0:
                        emit_mm1(p - 1, es, prev, None)
                    if mm2 is not None:
                        for _ in range(2):
                            next(mm2, None)
                    for _ in ts:
                        pass
                if mm2 is not None and not boundary:
                    for _ in range(2):
                        next(mm2, None)
                prev = qxt
            emit_mm1(npr - 1, es, prev, None)
            if mm2 is not None:
                for _ in mm2:
                    pass
            sm = [emit_softmax(i, es) for i in range(nct)]
            return xbs, x8, sm

        mm2 = None
        for b in range(bs):
            xbs, x8, sm = emit_batch_front(b, mm2)
            mm2 = mm2_stream(b, sm, x8, xbs)
        for _ in mm2:
            pass

    nc.compile()
    return nc


_NC_CACHE = {}


def _get_nc(key=(BS, C, N)):
    if key not in _NC_CACHE:
        _NC_CACHE[key] = build_nc(*key)
    return _NC_CACHE[key]


def _run(x, x_RGB, delta, trace=False):
    x = np.ascontiguousarray(np.asarray(x, dtype=np.float32)).reshape(B, C, N)
    xr = np.ascontiguousarray(np.asarray(x_RGB, dtype=np.float32)).reshape(B, C, N)
    d = np.asarray(delta, dtype=np.float32).reshape(-1)[0]
    d_b = np.full((128, 1), d, dtype=np.float32)

    nc = _get_nc()
    in_maps = []
    for cid in range(N_CORES):
        sl = slice(cid * BS, (cid + 1) * BS)
        in_maps.append(
            {
                "x": np.ascontiguousarray(x[sl]),
                "x_RGB": np.ascontiguousarray(xr[sl]),
                "delta": d_b,
            }
        )
    res = run_bass_kernel_spmd(nc, in_maps, core_ids=list(range(N_CORES)), trace=trace)
    out = np.concatenate(
        [np.asarray(r["out"]).astype(np.float32) for r in res.results], axis=0
    )
    return out.reshape(B, C, H, W), res


def kernel(x, x_RGB, delta):
    out, _ = _run(x, x_RGB, delta, trace=False)
    return out


# revision 5
# speedup vs baseline: 1.1491x; 1.1491x over previous
"""Trainium2 Bass kernel for the MFCA channel-attention module.

  q = x_RGB.reshape(B, C, N); k = v = x.reshape(B, C, N)
  energy    = q @ k^T                          (B, C, C)
  attention = softmax(max(energy, -1) - energy)   over last axis
  out       = delta * (attention @ v) + x

Numerically, softmax(max - energy) == softmax(-energy); the stable form is
p = exp(min_row(energy) - energy), attention = p / rowsum(p).

Sharding: data-parallel over batch B=16 across 8 NeuronCores (2 per core).

v7 (arrival-ordered PE stream at quarter cadence, split epilogue):
  - loads stay [128, 1024] quarters (4 KiB descriptors -- finer pieces
    measurably inflate DMA ring time by ~15%), quarter-major so arrival
    order matches transpose consumption order.
  - PE emission is strictly arrival-ordered: at a quarter boundary all
    ready work (MM1(p-1), MM2 fillers of the previous batch) is emitted
    BEFORE T(p), so the in-order PE queue never head-of-line-blocks on a
    load.  Within a quarter, MM1(p-1) interleaves into T(p).
  - MM2 epilogue split: ACT drains u PSUM->SBUF (bf16), DVE adds the two
    bf16 tensors at 2x rate.  Frees u banks earlier and halves the DVE
    cost that previously paced the tail at 99% DVE busy.
  - tail MM2 (last batch) draws u tiles from both PSUM pools (es banks
    are free after softmax) for a 4-deep matmul pipeline; stores stream
    per (i, 1024-column) chunk.
"""

from contextlib import ExitStack

import numpy as np

import concourse.bass as bass
import concourse.tile as tile
from concourse import bacc, mybir
from concourse.bass_utils import run_bass_kernel_spmd
from concourse.masks import make_identity

N_CORES = 8
B, C, H, W = 16, 512, 64, 64
N = H * W  # 4096
BS = B // N_CORES  # batches per core

F32 = mybir.dt.float32
BF16 = mybir.dt.bfloat16
FP8 = mybir.dt.float8e4

DR = mybir.MatmulPerfMode.DoubleRow


def build_nc(bs=BS, c=C, n=N):
    """Build the single-core Bass program (SPMD across all cores)."""
    nc = bacc.Bacc(None, target_bir_lowering=False, debug=False)

    x_d = nc.dram_tensor("x", [bs, c, n], F32, kind="ExternalInput")
    q_d = nc.dram_tensor("x_RGB", [bs, c, n], F32, kind="ExternalInput")
    d_d = nc.dram_tensor("delta", [128, 1], F32, kind="ExternalInput")
    o_d = nc.dram_tensor("out", [bs, c, n], BF16, kind="ExternalOutput")

    nct = c // 128  # channel chunks (i-tiles / j-tiles)
    npr = n // 256  # DoubleRow n-pairs (16)
    nnb = n // 512  # n-blocks in the output matmul (8)
    PC = 1024  # load piece columns (4 KiB descriptors)
    npc = n // PC  # pieces per chunk row (4)

    with tile.TileContext(nc) as tc, ExitStack() as ctx:
        pxb = ctx.enter_context(tc.tile_pool(name="pxb", bufs=34))
        px8 = ctx.enter_context(tc.tile_pool(name="px8", bufs=2))
        pq8 = ctx.enter_context(tc.tile_pool(name="pq8", bufs=2))
        pqt = ctx.enter_context(tc.tile_pool(name="pqt", bufs=4))
        pp = ctx.enter_context(tc.tile_pool(name="pp", bufs=6))
        ppt = ctx.enter_context(tc.tile_pool(name="ppt", bufs=5))
        pub = ctx.enter_context(tc.tile_pool(name="pub", bufs=6))
        pout = ctx.enter_context(tc.tile_pool(name="pout", bufs=8))
        psml = ctx.enter_context(tc.tile_pool(name="psml", bufs=8))
        pone = ctx.enter_context(tc.tile_pool(name="pone", bufs=1))
        pe_pool = ctx.enter_context(tc.tile_pool(name="pe", bufs=4, space="PSUM"))
        ptr_pool = ctx.enter_context(tc.tile_pool(name="ptr", bufs=2, space="PSUM"))
        pu_pool = ctx.enter_context(tc.tile_pool(name="pu", bufs=2, space="PSUM"))

        ident8 = pone.tile([128, 128], FP8)
        make_identity(nc, ident8[:])
        delta_sb = pone.tile([128, 1], F32)
        nc.sync.dma_start(out=delta_sb[:], in_=d_d[:])

        def emit_loads(b):
            """Load one batch in [128, PC] quarters, quarter-major, so data
            arrives in the order the transpose stream consumes it.  x lands
            as bf16 (residual + fp8 cast source), x_RGB as fp8 direct."""
            xbs = [[None] * npc for _ in range(nct)]
            x8 = px8.tile([128, nct, n], FP8, name="x8", tag="x8")
            q8 = pq8.tile([128, nct, n], FP8, name="q8", tag="q8")

            for pi in range(npc):
                cs = slice(pi * PC, (pi + 1) * PC)
                for k in range(nct):
                    xb = pxb.tile([128, PC], BF16)
                    nc.gpsimd.dma_start(
                        out=xb[:], in_=x_d[b, 128 * k : 128 * (k + 1), cs]
                    )
                    nc.gpsimd.dma_start(
                        out=q8[:, k, cs], in_=q_d[b, 128 * k : 128 * (k + 1), cs]
                    )
                    if (pi * nct + k) % 2 == 0:
                        nc.scalar.copy(out=x8[:, k, cs], in_=xb[:])
                    else:
                        nc.vector.tensor_copy(out=x8[:, k, cs], in_=xb[:])
                    xbs[k][pi] = xb
            return xbs, x8, q8

        def _sl(xbs, cc, c0, w):
            """Slice [c0, c0+w) of chunk cc out of per-quarter tiles."""
            h = c0 // PC
            return xbs[cc][h][:, c0 - h * PC : c0 - h * PC + w]

        def t_stream(p, nxt, q8, x8):
            """Generator yielding after each fp8 PE transpose of pair p, so
            the caller can interleave MM1 matmuls of the previous pair."""
            for li in range(2):
                nt = 2 * p + li
                ns = slice(128 * nt, 128 * (nt + 1))
                stage = ptr_pool.tile(
                    [128, 2, c, 2], FP8, name="tstage", tag="stage"
                )
                for cc in range(nct):
                    nc.tensor.transpose(
                        stage[:, 0, 128 * cc : 128 * (cc + 1), 0],
                        q8[:, cc, ns],
                        ident8[:],
                    )
                    yield
                    nc.tensor.transpose(
                        stage[:, 1, 128 * cc : 128 * (cc + 1), 0],
                        x8[:, cc, ns],
                        ident8[:],
                    )
                    yield
                if nt % 2 == 0:
                    nc.vector.tensor_copy(out=nxt[:, li, :, :], in_=stage[:, :, :, 0])
                else:
                    nc.scalar.copy(out=nxt[:, li, :, :], in_=stage[:, :, :, 0])

        def emit_mm1(p, es, qxt, ts):
            """4 DoubleRow matmuls (one per i-tile) for n-pair p; if ts is
            given, interleave the next pair's transposes 4-per-matmul."""
            for i in range(nct):
                nc.tensor.matmul(
                    es[i][:],
                    qxt[:, :, 0, 128 * i : 128 * (i + 1)],
                    qxt[:, :, 1, :],
                    start=(p == 0),
                    stop=(p == npr - 1),
                    perf_mode=DR,
                )
                if ts is not None:
                    for _ in range(4):
                        next(ts, None)
            if ts is not None:
                for _ in ts:
                    pass

        def emit_softmax(i, es):
            e = es[i]
            m = psml.tile([128, 1], F32)
            nc.vector.tensor_reduce(
                m[:], e[:], axis=mybir.AxisListType.X, op=mybir.AluOpType.min
            )
            p_t = pp.tile([128, c], BF16)
            z = psml.tile([128, 1], F32)
            nc.scalar.activation(
                out=p_t[:],
                in_=e[:],
                func=mybir.ActivationFunctionType.Exp,
                bias=m[:],
                scale=-1.0,
                accum_out=z[:],
            )
            zi = psml.tile([128, 1], F32)
            nc.vector.reciprocal(zi[:], z[:])
            s = psml.tile([128, 1], F32)
            nc.vector.tensor_scalar_mul(s[:], zi[:], delta_sb[:])  # delta / Z
            # Fold delta/Z into P here so MM2 needs no per-block scaling.
            ps = pp.tile([128, c], FP8)
            nc.vector.tensor_scalar_mul(ps[:], p_t[:], s[:])
            # P'^T via fp8 PE transposes (step-2 stage), drained to
            # [128, jt, 128] so the MM2 DoubleRow stationary is a jt-pair
            # slice.
            pstage = ptr_pool.tile(
                [128, nct, 128, 2], FP8, name="pstage", tag="stage"
            )
            for jt in range(nct):
                nc.tensor.transpose(
                    pstage[:, jt, :, 0],
                    ps[:, 128 * jt : 128 * (jt + 1)],
                    ident8[:],
                )
            pt = ppt.tile([128, nct, 128], FP8)
            nc.scalar.copy(out=pt[:], in_=pstage[:, :, :, 0])
            return pt

        def mm2_stream(b, sm, x8, xbs, upools):
            """Generator of MM2 blocks, i-major.  Per block: 2 DoubleRow
            matmuls into u (PSUM), ACT drain-cast u -> bf16 SBUF, DVE bf16
            add with the residual; stores stream per (i, 1024-col) chunk."""
            nu = len(upools)
            ublk = 0
            for i in range(nct):
                pt = sm[i]
                for nbp in range(nnb // 2):
                    ob = pout.tile([128, 1024], BF16, name=f"ob{nbp}_{i}", tag="ob")
                    for s in range(2):
                        gnb = 2 * nbp + s
                        ns = slice(512 * gnb, 512 * (gnb + 1))
                        upool, utag = upools[ublk % nu]
                        u = upool.tile([128, 512], F32, name="u", tag=utag)
                        ublk += 1
                        for jp in range(2):
                            nc.tensor.matmul(
                                u[:],
                                pt[:, 2 * jp : 2 * jp + 2, :],
                                x8[:, 2 * jp : 2 * jp + 2, ns],
                                start=(jp == 0),
                                stop=(jp == 1),
                                perf_mode=DR,
                            )
                        ub = pub.tile([128, 512], BF16, name="ub", tag="ub")
                        nc.scalar.copy(out=ub[:], in_=u[:])
                        nc.vector.tensor_add(
                            ob[:, 512 * s : 512 * (s + 1)],
                            ub[:],
                            _sl(xbs, i, 512 * gnb, 512),
                        )
                        yield
                    nc.sync.dma_start(
                        out=o_d[
                            b, 128 * i : 128 * (i + 1), 1024 * nbp : 1024 * (nbp + 1)
                        ],
                        in_=ob[:],
                    )

        def emit_batch_front(b, mm2):
            """Loads, transposes, energy matmuls, and softmax for one batch;
            the previous batch's MM2 blocks fill load-stall gaps.  Emission
            is strictly arrival-ordered: at a quarter boundary all ready
            work (MM1(p-1), fillers) is emitted BEFORE T(p) so the in-order
            PE queue never blocks on a load."""
            xbs, x8, q8 = emit_loads(b)
            es = [
                pe_pool.tile([128, c], F32, name=f"e{i}", tag="e") for i in range(nct)
            ]
            prev = None
            for p in range(npr):
                qxt = pqt.tile([128, 2, 2, c], FP8, name="qxt", tag="qxt")
                ts = t_stream(p, qxt, q8, x8)
                boundary = (p * 256) % PC == 0  # pair p starts a new quarter
                if p > 0 and not boundary:
                    emit_mm1(p - 1, es, prev, ts)  # interleave into T(p)
                else:
                    if p > 0:
                        emit_mm1(p - 1, es, prev, None)
                    if mm2 is not None:
                        for _ in range(2):
                            next(mm2, None)
                    for _ in ts:
                        pass
                if mm2 is not None and not boundary:
                    for _ in range(2):
                        next(mm2, None)
                prev = qxt
            emit_mm1(npr - 1, es, prev, None)
            if mm2 is not None:
                for _ in mm2:
                    pass
            sm = [emit_softmax(i, es) for i in range(nct)]
            return xbs, x8, sm

        mm2 = None
        for b in range(bs):
            xbs, x8, sm = emit_batch_front(b, mm2)
            tail = b == bs - 1
            upools = (
                [(pu_pool, "u"), (pu_pool, "u"), (pe_pool, "e"), (pe_pool, "e")]
                if tail
                else [(pu_pool, "u"), (pu_pool, "u")]
            )
            mm2 = mm2_stream(b, sm, x8, xbs, upools)
        for _ in mm2:
            pass

    nc.compile()
    return nc


_NC_CACHE = {}


def _get_nc(key=(BS, C, N)):
    if key not in _NC_CACHE:
        _NC_CACHE[key] = build_nc(*key)
    return _NC_CACHE[key]


def _run(x, x_RGB, delta, trace=False):
    x = np.ascontiguousarray(np.asarray(x, dtype=np.float32)).reshape(B, C, N)
    xr = np.ascontiguousarray(np.asarray(x_RGB, dtype=np.float32)).reshape(B, C, N)
    d = np.asarray(delta, dtype=np.float32).reshape(-1)[0]
    d_b = np.full((128, 1), d, dtype=np.float32)

    nc = _get_nc()
    in_maps = []
    for cid in range(N_CORES):
        sl = slice(cid * BS, (cid + 1) * BS)
        in_maps.append(
            {
                "x": np.ascontiguousarray(x[sl]),
                "x_RGB": np.ascontiguousarray(xr[sl]),
                "delta": d_b,
            }
        )
    res = run_bass_kernel_spmd(nc, in_maps, core_ids=list(range(N_CORES)), trace=trace)
    out = np.concatenate(
        [np.asarray(r["out"]).astype(np.float32) for r in res.results], axis=0
    )
    return out.reshape(B, C, H, W), res


def kernel(x, x_RGB, delta):
    out, _ = _run(x, x_RGB, delta, trace=False)
    return out
